# revision 3
# baseline (speedup 1.0000x reference)
"""Trainium2 Bass kernel for nn_CustomConvLayer (bilinear-tap conv).

Math: each of the K=9 taps gathers x at constant sub-pixel offset
(dy, dx) via separable bilinear interpolation, scales by a per-
(cout, cin, tap) weight, and accumulates over taps and input channels.

Fast path (all taps share the same fractional offset, as in the
reference's base+0.4 grid): the op factors exactly into
    out = conv_int(blend_x(blend_y(xp)), W3)
where blend_y/x are the shared 2-tap bilinear blends applied once to
the padded input and W3[o,i,iy,ix] = sum_k w[o,i,k] at the integer tap
positions (x the blend normalization). The 3x3 conv runs as 6 banded
128x128 TensorE blocks (75% dense -- provably minimal for the
row-pair M/K packing), bf16.

Distribution: data-parallel over batch, one image per NeuronCore.

Pipeline (v2):
  - SBUF layout: even padded rows on partitions 0-63, odd rows on
    64-127; segments of row pairs x 132 cols; host packs the exact
    bf16 SBUF image (zeros baked in).
  - ship=1: ONE input copy; the y-blend reads its two source rows via
    cross-partition-base operands (rows 2t/2t+1 at partition halves).
  - ship=2: TWO row-aligned input copies (baseline scheme); y-blend is
    lane-local full-width.
  - Input streams in small chunks; blends chase the DMA, matmuls chase
    the blends in 2-sg groups, drains (Act [+Pool]) and output DMAs
    chase the matmuls.  A couple of dummy matmuls warm the PE p-state
    while the first chunk loads.
"""

import os
import numpy as np
import ml_dtypes

import concourse.bass as bass  # noqa: F401
import concourse.mybir as mybir
import concourse.tile as tile
from concourse import bacc
from concourse.bass_utils import run_bass_kernel_spmd

B, CIN, H, W = 8, 64, 128, 128
COUT, KTAPS = 64, 9
NCORES = 8

WP = 132          # padded segment width
NSEG = 65         # row-pair segments holding blended rows
BF16 = ml_dtypes.bfloat16

# --- tuning knobs (env overrides for experiments; defaults = best) ---
SHIP = int(os.environ.get("CONV_SHIP", "2"))
WARMUP = int(os.environ.get("CONV_WARMUP", "2"))
POOL_DRAIN = int(os.environ.get("CONV_POOL_DRAIN", "1"))
CHUNKS_ENV = os.environ.get("CONV_CHUNKS", "")


# --------------------------------------------------------------------
# planning (host)
# --------------------------------------------------------------------

def fold_weights(weights, tap_offsets):
    """General path: fold per-tap scalar weights + bilinear coeffs into
    W_eff [COUT, CIN, Fy, Fx] (float64)."""
    w = np.asarray(weights, np.float64)
    off = np.asarray(tap_offsets, np.float64)
    dy, dx = off[:, 0], off[:, 1]
    assert (dy >= 0).all() and (dx >= 0).all(), "negative tap offsets unsupported"
    iy = np.floor(dy).astype(np.int64)
    fy = dy - iy
    ix = np.floor(dx).astype(np.int64)
    fx = dx - ix
    Fy = int(iy.max()) + 2
    Fx = int(ix.max()) + 2
    assert Fy <= 5 and Fx <= 5
    Weff = np.zeros((COUT, CIN, Fy, Fx))
    for k in range(KTAPS):
        for a, cy in ((0, 1.0 - fy[k]), (1, fy[k])):
            for bb, cx in ((0, 1.0 - fx[k]), (1, fx[k])):
                Weff[:, :, iy[k] + a, ix[k] + bb] += w[:, :, k] * (cy * cx)
    return Weff


def make_blocks(Weff):
    """Build the (delta, v) lhsT blocks of the banded row-pair matmul
    structure.

    Block (delta, v) couples input row-pair t = 4*sg + g + delta to
    output row-pair 4*sg + g:  lhsT[(j, ci), (i, co)] = W_eff[co, ci,
    u = 2*delta + j - i, v] (zero when u out of range).

    Returns (blocks, Wh): blocks is a list of (delta, v); Wh is
    [128, nblk*128] float64 with block bi at columns bi*128:(bi+1)*128.
    """
    _, _, Fy, Fx = Weff.shape
    ndelta = Fy // 2 + 1
    blocks, mats = [], []
    for d in range(ndelta):
        for v in range(Fx):
            Mb = np.zeros((128, 128))
            nz = False
            for j in (0, 1):
                for i in (0, 1):
                    u = 2 * d + j - i
                    if 0 <= u < Fy:
                        blk = Weff[:, :, u, v].T  # [cin, cout]
                        Mb[j * 64:(j + 1) * 64, i * 64:(i + 1) * 64] = blk
                        nz = nz or bool(np.abs(blk).max() > 0)
            if nz:
                blocks.append((d, v))
                mats.append(Mb)
    Wh = np.stack(mats, 0).transpose(1, 0, 2).reshape(128, -1)
    return blocks, np.ascontiguousarray(Wh)


def plan_from_inputs(weights, tap_offsets):
    """Decide fast vs general path and precompute the weight blocks."""
    w = np.asarray(weights, np.float64)
    off = np.asarray(tap_offsets, np.float64)
    dy, dx = off[:, 0], off[:, 1]
    iy = np.floor(dy).astype(np.int64)
    fy = dy - iy
    ix = np.floor(dx).astype(np.int64)
    fx = dx - ix
    fast = (
        (dy >= 0).all() and (dx >= 0).all()
        and float(np.ptp(fy)) < 1e-5 and float(np.ptp(fx)) < 1e-5
        and int(iy.max()) <= 2 and int(ix.max()) <= 2
    )
    if not fast:
        Weff = fold_weights(weights, tap_offsets)
        blocks, Wh = make_blocks(Weff)
        return {
            "mode": "general",
            "blocks": blocks,
            "Wh": Wh,
            "key": ("general", tuple(blocks)),
        }

    fy0, fx0 = float(fy.mean()), float(fx.mean())
    # role flags: blend = base + s * scaled, with the larger bilinear
    # coefficient on the unscaled operand (keeps s <= 1)
    y_swap = fy0 > 0.5          # unscaled operand is row r+1
    x_swap = fx0 > 0.5          # unscaled operand is col c+1
    sy = fy0 / (1.0 - fy0) if not y_swap else (1.0 - fy0) / fy0
    sx = fx0 / (1.0 - fx0) if not x_swap else (1.0 - fx0) / fx0
    scale = ((1.0 - fy0) if not y_swap else fy0) * (
        (1.0 - fx0) if not x_swap else fx0)

    Fy = int(iy.max()) + 1
    Fx = int(ix.max()) + 1
    W3 = np.zeros((COUT, CIN, Fy, Fx))
    for k in range(KTAPS):
        W3[:, :, iy[k], ix[k]] += w[:, :, k]
    W3 *= scale
    blocks, Wh = make_blocks(W3)
    return {
        "mode": "fast",
        "blocks": blocks,
        "Wh": Wh,
        "sy": sy,
        "sx": sx,
        "y_swap": y_swap,
        "x_swap": x_swap,
        "key": ("fast", tuple(blocks), round(sy, 9), round(sx, 9),
                y_swap, x_swap),
    }


def _chunk_ends(ship):
    if CHUNKS_ENV:
        ends = [int(t) for t in CHUNKS_ENV.split(",")]
        return ends
    if ship == 1:
        return [4, 8, 12, 18, 26, 34, 42, 50, 58, 66]
    return [4, 8, 12, 18, 26, 34, 42, 50, 58, 65]


# --------------------------------------------------------------------
# device program
# --------------------------------------------------------------------

def build_fast(plan, repeat=1, ship=None, warmup=None, pool_drain=None):
    """3x3 integer conv (6 banded blocks, bf16) + on-chip bilinear
    blend, software-pipelined: chunked loads -> blends -> 2-sg matmul
    groups -> split drains -> streamed output."""
    if ship is None:
        ship = SHIP
    if warmup is None:
        warmup = WARMUP
    if pool_drain is None:
        pool_drain = POOL_DRAIN
    blocks = plan["blocks"]
    nblk = len(blocks)
    sy, sx = plan["sy"], plan["sx"]
    y_swap, x_swap = plan["y_swap"], plan["x_swap"]
    dt = mybir.dt.bfloat16
    nseg_in = 66 if ship == 1 else NSEG
    ends = _chunk_ends(ship)
    assert ends[-1] == nseg_in

    nc = bacc.Bacc(
        "TRN2", target_bir_lowering=False, debug=False, enable_asserts=False
    )
    xa = nc.dram_tensor("xa", [128, nseg_in * WP], dt, kind="ExternalInput")
    if ship == 2:
        xb2 = nc.dram_tensor("xb2", [128, NSEG * WP], dt, kind="ExternalInput")
    wb = nc.dram_tensor("wb", [128, nblk * 128], dt, kind="ExternalInput")
    out = nc.dram_tensor("out", [128, 16 * 512], dt, kind="ExternalOutput")

    with tile.TileContext(nc) as tc:
        with (
            tc.tile_pool(name="const", bufs=1) as const_pool,
            tc.tile_pool(name="big", bufs=2) as big_pool,
            tc.tile_pool(name="scr", bufs=3) as scr_pool,
            tc.tile_pool(name="psum", bufs=5, space="PSUM") as psum_pool,
            tc.tile_pool(name="wpsum", bufs=1, space="PSUM") as wpsum_pool,
            tc.tile_pool(name="stage", bufs=3) as stage_pool,
        ):
            wbuf = const_pool.tile([128, nblk * 128], dt, tag="wbuf")
            xav = xa.ap().rearrange("p (t c) -> p t c", c=WP)
            if ship == 2:
                xbv = xb2.ap().rearrange("p (t c) -> p t c", c=WP)
            outv = out.ap().rearrange("p (s n) -> p s n", n=512)

            for _rep in range(repeat):
                abuf = big_pool.tile([128, nseg_in, WP], dt, tag="abuf",
                                     name=f"abuf_{_rep}")
                if ship == 2:
                    bbuf = big_pool.tile([128, NSEG, WP], dt, tag="bbuf",
                                         name=f"bbuf_{_rep}")
                rybuf = big_pool.tile([128, NSEG, WP], dt, tag="rybuf",
                                      name=f"rybuf_{_rep}")
                xbuf = big_pool.tile([128, NSEG, WP], dt, tag="xbuf",
                                     name=f"xbuf_{_rep}")

                if _rep == 0:
                    nc.sync.dma_start(out=wbuf[:], in_=wb.ap())
                    if warmup:
                        wt = wpsum_pool.tile([128, 512], mybir.dt.float32,
                                             tag="wt")
                        for _wi in range(warmup):
                            nc.tensor.matmul(
                                wt[:], wbuf[:, 0:128], wbuf[:, 0:512],
                                start=True, stop=True,
                            )

                # streaming state
                even_done = 0   # rybuf even segs completed (exclusive)
                odd_done = 0
                x_done = 0
                group = 0       # next matmul group to issue (8 groups of 2 sg)

                def issue_group(g):
                    ptiles = []
                    for si in (0, 1):
                        sg = 2 * g + si
                        pt = psum_pool.tile(
                            [128, 4, 128], mybir.dt.float32, tag="ps",
                            name=f"ps_{_rep}_{sg}",
                        )
                        ptiles.append(pt)
                        for bi, (d, v) in enumerate(blocks):
                            lhsT = wbuf[:, bi * 128:(bi + 1) * 128]
                            t0 = 4 * sg + d
                            rhs = xbuf[:, t0:t0 + 4, v:v + 128]
                            nc.tensor.matmul(
                                pt[:], lhsT, rhs,
                                start=(bi == 0), stop=(bi == nblk - 1),
                            )
                    st = stage_pool.tile([128, 2, 512], dt, tag="st",
                                         name=f"st_{_rep}_{g}")
                    nc.scalar.copy(
                        out=st[:, 0],
                        in_=ptiles[0][:].rearrange("p a b -> p (a b)"))
                    if pool_drain:
                        nc.vector.tensor_copy(
                            st[:, 1],
                            ptiles[1][:].rearrange("p a b -> p (a b)"))
                    else:
                        nc.scalar.copy(
                            out=st[:, 1],
                            in_=ptiles[1][:].rearrange("p a b -> p (a b)"))
                    nc.gpsimd.dma_start(
                        out=outv[:, 2 * g:2 * g + 2], in_=st[:])

                c0 = 0
                for ci, c1 in enumerate(ends):
                    # ---- load chunk ----
                    nc.sync.dma_start(out=abuf[:, c0:c1], in_=xav[:, c0:c1])
                    if ship == 2:
                        nc.sync.dma_start(out=bbuf[:, c0:c1],
                                          in_=xbv[:, c0:c1])

                    # ---- y-blend ----
                    if ship == 1:
                        e0, e1 = even_done, min(c1, NSEG)
                        o0, o1 = odd_done, c1 - 1
                        ln = max(e1 - e0, o1 - o0)
                        scr = scr_pool.tile([128, max(ln, 1), WP], dt,
                                            tag="scry",
                                            name=f"scry_{_rep}_{ci}")
                        if e1 > e0:
                            lo, hi = e0, e1
                            base = abuf[0:64, lo:hi]
                            sc = abuf[64:128, lo:hi]
                            if y_swap:
                                base, sc = sc, base
                            nc.scalar.activation(
                                scr[0:64, 0:hi - lo], sc,
                                mybir.ActivationFunctionType.Copy,
                                scale=float(sy))
                            nc.vector.tensor_tensor(
                                rybuf[0:64, lo:hi], scr[0:64, 0:hi - lo],
                                base, op=mybir.AluOpType.add)
                            even_done = e1
                        if o1 > o0:
                            lo, hi = o0, o1
                            base = abuf[64:128, lo:hi]
                            sc = abuf[0:64, lo + 1:hi + 1]
                            if y_swap:
                                base, sc = sc, base
                            nc.vector.tensor_scalar(
                                scr[64:128, 0:hi - lo], sc, float(sy), None,
                                op0=mybir.AluOpType.mult)
                            nc.vector.tensor_tensor(
                                rybuf[64:128, lo:hi], scr[64:128, 0:hi - lo],
                                base, op=mybir.AluOpType.add)
                            odd_done = o1
                        y_done = min(even_done, odd_done)
                    else:
                        lo, hi = even_done, c1
                        ln = hi - lo
                        scr = scr_pool.tile([128, max(ln, 1), WP], dt,
                                            tag="scry",
                                            name=f"scry_{_rep}_{ci}")
                        ya, yb = abuf[:, lo:hi], bbuf[:, lo:hi]
                        y_sc, y_base = (ya, yb) if y_swap else (yb, ya)
                        nc.scalar.activation(
                            scr[:, 0:ln], y_sc,
                            mybir.ActivationFunctionType.Copy,
                            scale=float(sy))
                        nc.vector.tensor_tensor(
                            rybuf[:, lo:hi], scr[:, 0:ln], y_base,
                            op=mybir.AluOpType.add)
                        even_done = odd_done = c1
                        y_done = c1

                    # ---- x-blend ----
                    if y_done > x_done:
                        lo, hi = x_done, y_done
                        ln = hi - lo
                        scrx = scr_pool.tile([128, ln, 131], dt, tag="scrx",
                                             name=f"scrx_{_rep}_{ci}")
                        r0 = rybuf[:, lo:hi, 0:131]
                        r1 = rybuf[:, lo:hi, 1:132]
                        x_sc, x_base = (r0, r1) if x_swap else (r1, r0)
                        nc.vector.tensor_scalar(
                            scrx[:], x_sc, float(sx), None,
                            op0=mybir.AluOpType.mult)
                        nc.vector.tensor_tensor(
                            xbuf[:, lo:hi, 0:131], scrx[:], x_base,
                            op=mybir.AluOpType.add)
                        x_done = y_done

                    # ---- matmul groups now unlocked ----
                    while group < 8 and x_done >= 8 * group + 9:
                        issue_group(group)
                        group += 1
                while group < 8:
                    issue_group(group)
                    group += 1

    nc.compile()
    return nc


def build_general(plan, repeat=1):
    """Fallback: dense folded conv (up to 12 banded blocks), f32r,
    device-side padding memsets."""
    blocks = plan["blocks"]
    nblk = len(blocks)
    dt_x = mybir.dt.float32r
    ms_cast = lambda ap: ap.bitcast(mybir.dt.float32)  # noqa: E731
    GWP, GNSEG = 132, 66
    nc = bacc.Bacc(
        "TRN2", target_bir_lowering=False, debug=False, enable_asserts=False
    )
    xs = nc.dram_tensor("xs", [CIN, H, W], dt_x, kind="ExternalInput")
    wb = nc.dram_tensor("wb", [128, nblk * 128], dt_x, kind="ExternalInput")
    out = nc.dram_tensor("out", [COUT, H, W], mybir.dt.float32,
                         kind="ExternalOutput")

    with tile.TileContext(nc) as tc:
        with (
            tc.tile_pool(name="const", bufs=1) as const_pool,
            tc.tile_pool(name="psum", bufs=8, space="PSUM") as psum_pool,
            tc.tile_pool(name="stage", bufs=8) as stage_pool,
        ):
            xbuf = const_pool.tile([128, GNSEG * GWP], dt_x, tag="xbuf")
            wbuf = const_pool.tile([128, nblk * 128], dt_x, tag="wbuf")
            xv = xbuf[:].rearrange("p (t c) -> p t c", c=GWP)

            for _rep in range(repeat):
                nc.sync.dma_start(out=wbuf[:], in_=wb.ap())

                nc.gpsimd.memset(ms_cast(xv[:, :, 0:1]), 0.0)
                nc.gpsimd.memset(ms_cast(xv[:, :, 129:132]), 0.0)
                nc.gpsimd.memset(ms_cast(xv[0:64, 0:1, :]), 0.0)
                nc.gpsimd.memset(ms_cast(xv[64:128, 64:65, :]), 0.0)
                nc.gpsimd.memset(ms_cast(xv[:, 65:66, :]), 0.0)

                xap = xs.ap()
                for t0, t1 in ((1, 17), (17, 33), (33, 49), (49, 64)):
                    dram = xap[:, 2 * t0 - 1:2 * t1 - 1, :].rearrange(
                        "ci (t j) x -> j ci t x", j=2
                    )
                    for j in (0, 1):
                        nc.sync.dma_start(
                            out=xv[j * 64:(j + 1) * 64, t0:t1, 1:129],
                            in_=dram[j],
                        )
                nc.sync.dma_start(out=xv[64:128, 0:1, 1:129], in_=xap[:, 0:1, :])
                nc.sync.dma_start(out=xv[0:64, 64:65, 1:129],
                                  in_=xap[:, 127:128, :])

                out_ap = out.ap().rearrange(
                    "co (s g i) x -> s i co g x", g=4, i=2)

                for half in range(2):
                    ptiles = [
                        psum_pool.tile(
                            [128, 4, 128], mybir.dt.float32, tag="ps",
                            name=f"ps_{_rep}_{half}_{k}",
                        )
                        for k in range(8)
                    ]
                    for bi, (d, v) in enumerate(blocks):
                        lhsT = wbuf[:, bi * 128:(bi + 1) * 128]
                        first = bi == 0
                        last = bi == nblk - 1
                        for sg in range(half * 8, half * 8 + 8):
                            t0 = 4 * sg + d
                            rhs = xv[:, t0:t0 + 4, v:v + 128]
                            nc.tensor.matmul(
                                ptiles[sg % 8][:], lhsT, rhs,
                                start=first, stop=last,
                            )
                    for sg in range(half * 8, half * 8 + 8):
                        st = stage_pool.tile(
                            [128, 4, 128], mybir.dt.float32, tag="st",
                            name=f"st_{_rep}_{half}_{sg}",
                        )
                        nc.vector.tensor_copy(st[:], ptiles[sg % 8][:])
                        for i in (0, 1):
                            nc.sync.dma_start(
                                out=out_ap[sg][i],
                                in_=st[i * 64:(i + 1) * 64],
                            )

    nc.compile()
    return nc


_CACHE = {}


def _get_nc(plan, repeat=1):
    key = (plan["key"], repeat, SHIP, WARMUP, POOL_DRAIN, CHUNKS_ENV)
    if key not in _CACHE:
        builder = build_fast if plan["mode"] == "fast" else build_general
        _CACHE[key] = builder(plan, repeat)
    return _CACHE[key]


# --------------------------------------------------------------------
# host-side staging
# --------------------------------------------------------------------

def make_in_maps(x, plan):
    """Per-core input dicts. Fast path: pack each image into the
    row-pair SBUF layout(s) (bf16, zeros baked in)."""
    x = np.asarray(x)
    if plan["mode"] == "general":
        Whc = np.ascontiguousarray(plan["Wh"].astype(np.float32))
        return [
            {"xs": np.ascontiguousarray(x[b].astype(np.float32)), "wb": Whc}
            for b in range(B)
        ]
    Whc = np.ascontiguousarray(plan["Wh"].astype(BF16))
    xb16 = x.astype(BF16)
    in_maps = []
    for b in range(B):
        xp = np.zeros((CIN, 132, 132), dtype=BF16)
        xp[:, 1:129, 1:129] = xb16[b]
        if SHIP == 1:
            A = np.ascontiguousarray(
                xp.reshape(CIN, 66, 2, 132)
                .transpose(2, 0, 1, 3).reshape(128, 66 * WP))
            in_maps.append({"xa": A, "wb": Whc})
        else:
            A = np.ascontiguousarray(
                xp[:, 0:130, :].reshape(CIN, NSEG, 2, 132)
                .transpose(2, 0, 1, 3).reshape(128, NSEG * WP))
            Bm = np.ascontiguousarray(
                xp[:, 1:131, :].reshape(CIN, NSEG, 2, 132)
                .transpose(2, 0, 1, 3).reshape(128, NSEG * WP))
            in_maps.append({"xa": A, "xb2": Bm, "wb": Whc})
    return in_maps


def unpack_out(res, plan):
    """Per-core output dict -> [COUT, H, W] float32."""
    arr = np.asarray(res["out"])
    if plan["mode"] == "general":
        return arr.astype(np.float32)
    return np.ascontiguousarray(
        arr.reshape(2, 64, 16, 4, 128).transpose(1, 2, 3, 0, 4)
        .reshape(COUT, H, W)).astype(np.float32)


def kernel(x, weights, tap_offsets):
    x = np.asarray(x)
    assert x.shape == (B, CIN, H, W)
    plan = plan_from_inputs(weights, tap_offsets)
    nc = _get_nc(plan)
    in_maps = make_in_maps(x, plan)
    res = run_bass_kernel_spmd(nc, in_maps, list(range(NCORES)), trace=False)
    outs = [unpack_out(res.results[c], plan) for c in range(NCORES)]
    return np.stack(outs, 0).astype(np.float32)


# revision 8
# speedup vs baseline: 1.4231x; 1.4231x over previous
"""Trainium2 Bass kernel for nn_CustomConvLayer (bilinear-tap conv).

Math: each of the K=9 taps gathers x at constant sub-pixel offset
(dy, dx) via separable bilinear interpolation, scales by a per-
(cout, cin, tap) weight, and accumulates over taps and input channels.

Fast path (all taps share the same fractional offset, as in the
reference's base+0.4 grid): the op factors exactly into
    out = conv_int(blend_x(blend_y(xp)), W3)
where blend_y/x are the shared 2-tap bilinear blends applied once to
the padded input and W3[o,i,iy,ix] = sum_k w[o,i,k] at the integer tap
positions (x the blend normalization). The 3x3 conv runs as 6 banded
128x128 TensorE blocks (75% dense -- provably minimal for the
row-pair M/K packing), bf16.

Distribution: data-parallel over batch, one image per NeuronCore.

Pipeline (v2):
  - SBUF layout: even padded rows on partitions 0-63, odd rows on
    64-127; segments of row pairs x 132 cols; host packs the exact
    bf16 SBUF image (zeros baked in).
  - ship=1: ONE input copy; the y-blend reads its two source rows via
    cross-partition-base operands (rows 2t/2t+1 at partition halves).
  - ship=2: TWO row-aligned input copies (baseline scheme); y-blend is
    lane-local full-width.
  - Input streams in small chunks; blends chase the DMA, matmuls chase
    the blends in 2-sg groups, drains (Act [+Pool]) and output DMAs
    chase the matmuls.  A couple of dummy matmuls warm the PE p-state
    while the first chunk loads.
"""

import os
import numpy as np
import ml_dtypes

import concourse.bass as bass  # noqa: F401
import concourse.mybir as mybir
import concourse.tile as tile
from concourse import bacc
from concourse.bass_utils import run_bass_kernel_spmd

B, CIN, H, W = 8, 64, 128, 128
COUT, KTAPS = 64, 9
NCORES = 8

WP = 132          # padded segment width
NSEG = 65         # row-pair segments holding blended rows
BF16 = ml_dtypes.bfloat16

# --- tuning knobs (env overrides for experiments; defaults = best) ---
SHIP = int(os.environ.get("CONV_SHIP", "2"))
WARMUP = int(os.environ.get("CONV_WARMUP", "3"))
POOL_DRAIN = int(os.environ.get("CONV_POOL_DRAIN", "1"))
CHUNKS_ENV = os.environ.get("CONV_CHUNKS", "")


# --------------------------------------------------------------------
# planning (host)
# --------------------------------------------------------------------

def fold_weights(weights, tap_offsets):
    """General path: fold per-tap scalar weights + bilinear coeffs into
    W_eff [COUT, CIN, Fy, Fx] (float64)."""
    w = np.asarray(weights, np.float64)
    off = np.asarray(tap_offsets, np.float64)
    dy, dx = off[:, 0], off[:, 1]
    assert (dy >= 0).all() and (dx >= 0).all(), "negative tap offsets unsupported"
    iy = np.floor(dy).astype(np.int64)
    fy = dy - iy
    ix = np.floor(dx).astype(np.int64)
    fx = dx - ix
    Fy = int(iy.max()) + 2
    Fx = int(ix.max()) + 2
    assert Fy <= 5 and Fx <= 5
    Weff = np.zeros((COUT, CIN, Fy, Fx))
    for k in range(KTAPS):
        for a, cy in ((0, 1.0 - fy[k]), (1, fy[k])):
            for bb, cx in ((0, 1.0 - fx[k]), (1, fx[k])):
                Weff[:, :, iy[k] + a, ix[k] + bb] += w[:, :, k] * (cy * cx)
    return Weff


def make_blocks(Weff):
    """Build the (delta, v) lhsT blocks of the banded row-pair matmul
    structure.

    Block (delta, v) couples input row-pair t = 4*sg + g + delta to
    output row-pair 4*sg + g:  lhsT[(j, ci), (i, co)] = W_eff[co, ci,
    u = 2*delta + j - i, v] (zero when u out of range).

    Returns (blocks, Wh): blocks is a list of (delta, v); Wh is
    [128, nblk*128] float64 with block bi at columns bi*128:(bi+1)*128.
    """
    _, _, Fy, Fx = Weff.shape
    ndelta = Fy // 2 + 1
    blocks, mats = [], []
    for d in range(ndelta):
        for v in range(Fx):
            Mb = np.zeros((128, 128))
            nz = False
            for j in (0, 1):
                for i in (0, 1):
                    u = 2 * d + j - i
                    if 0 <= u < Fy:
                        blk = Weff[:, :, u, v].T  # [cin, cout]
                        Mb[j * 64:(j + 1) * 64, i * 64:(i + 1) * 64] = blk
                        nz = nz or bool(np.abs(blk).max() > 0)
            if nz:
                blocks.append((d, v))
                mats.append(Mb)
    Wh = np.stack(mats, 0).transpose(1, 0, 2).reshape(128, -1)
    return blocks, np.ascontiguousarray(Wh)


def plan_from_inputs(weights, tap_offsets):
    """Decide fast vs general path and precompute the weight blocks."""
    w = np.asarray(weights, np.float64)
    off = np.asarray(tap_offsets, np.float64)
    dy, dx = off[:, 0], off[:, 1]
    iy = np.floor(dy).astype(np.int64)
    fy = dy - iy
    ix = np.floor(dx).astype(np.int64)
    fx = dx - ix
    fast = (
        (dy >= 0).all() and (dx >= 0).all()
        and float(np.ptp(fy)) < 1e-5 and float(np.ptp(fx)) < 1e-5
        and int(iy.max()) <= 2 and int(ix.max()) <= 2
    )
    if not fast:
        Weff = fold_weights(weights, tap_offsets)
        blocks, Wh = make_blocks(Weff)
        return {
            "mode": "general",
            "blocks": blocks,
            "Wh": Wh,
            "key": ("general", tuple(blocks)),
        }

    fy0, fx0 = float(fy.mean()), float(fx.mean())
    # role flags: blend = base + s * scaled, with the larger bilinear
    # coefficient on the unscaled operand (keeps s <= 1)
    y_swap = fy0 > 0.5          # unscaled operand is row r+1
    x_swap = fx0 > 0.5          # unscaled operand is col c+1
    sy = fy0 / (1.0 - fy0) if not y_swap else (1.0 - fy0) / fy0
    sx = fx0 / (1.0 - fx0) if not x_swap else (1.0 - fx0) / fx0
    scale = ((1.0 - fy0) if not y_swap else fy0) * (
        (1.0 - fx0) if not x_swap else fx0)

    Fy = int(iy.max()) + 1
    Fx = int(ix.max()) + 1
    W3 = np.zeros((COUT, CIN, Fy, Fx))
    for k in range(KTAPS):
        W3[:, :, iy[k], ix[k]] += w[:, :, k]
    W3 *= scale
    blocks, Wh = make_blocks(W3)
    return {
        "mode": "fast",
        "blocks": blocks,
        "Wh": Wh,
        "sy": sy,
        "sx": sx,
        "y_swap": y_swap,
        "x_swap": x_swap,
        "key": ("fast", tuple(blocks), round(sy, 9), round(sx, 9),
                y_swap, x_swap),
    }


def _chunk_ends(ship):
    if CHUNKS_ENV:
        ends = [int(t) for t in CHUNKS_ENV.split(",")]
        return ends
    if ship == 1:
        return [4, 10, 18, 26, 34, 42, 50, 58, 66]
    return [4, 9, 17, 25, 33, 41, 49, 57, 65]


# --------------------------------------------------------------------
# device program
# --------------------------------------------------------------------

def build_fast(plan, repeat=1, ship=None, warmup=None, pool_drain=None):
    """3x3 integer conv (6 banded blocks, bf16) + on-chip bilinear
    blend, software-pipelined: chunked loads -> blends -> 2-sg matmul
    groups -> split drains -> streamed output."""
    if ship is None:
        ship = SHIP
    if warmup is None:
        warmup = WARMUP
    if pool_drain is None:
        pool_drain = POOL_DRAIN
    blocks = plan["blocks"]
    nblk = len(blocks)
    sy, sx = plan["sy"], plan["sx"]
    y_swap, x_swap = plan["y_swap"], plan["x_swap"]
    dt = mybir.dt.bfloat16
    nseg_in = 66 if ship == 1 else NSEG
    ends = _chunk_ends(ship)
    assert ends[-1] == nseg_in

    nc = bacc.Bacc(
        "TRN2", target_bir_lowering=False, debug=False, enable_asserts=False
    )
    xa = nc.dram_tensor("xa", [128, nseg_in * WP], dt, kind="ExternalInput")
    if ship == 2:
        xb2 = nc.dram_tensor("xb2", [128, NSEG * WP], dt, kind="ExternalInput")
    wb = nc.dram_tensor("wb", [128, nblk * 128], dt, kind="ExternalInput")
    out = nc.dram_tensor("out", [128, 16 * 512], dt, kind="ExternalOutput")

    with tile.TileContext(nc) as tc:
        with (
            tc.tile_pool(name="const", bufs=1) as const_pool,
            tc.tile_pool(name="big", bufs=2) as big_pool,
            tc.tile_pool(name="scr", bufs=4) as scr_pool,
            tc.tile_pool(name="psum", bufs=5, space="PSUM") as psum_pool,
            tc.tile_pool(name="wpsum", bufs=1, space="PSUM") as wpsum_pool,
            tc.tile_pool(name="stage", bufs=3) as stage_pool,
        ):
            wbuf = const_pool.tile([128, nblk * 128], dt, tag="wbuf")
            xav = xa.ap().rearrange("p (t c) -> p t c", c=WP)
            if ship == 2:
                xbv = xb2.ap().rearrange("p (t c) -> p t c", c=WP)
            outv = out.ap().rearrange("p (s n) -> p s n", n=512)

            for _rep in range(repeat):
                abuf = big_pool.tile([128, nseg_in, WP], dt, tag="abuf",
                                     name=f"abuf_{_rep}")
                if ship == 2:
                    bbuf = big_pool.tile([128, NSEG, WP], dt, tag="bbuf",
                                         name=f"bbuf_{_rep}")
                rybuf = big_pool.tile([128, NSEG, WP], dt, tag="rybuf",
                                      name=f"rybuf_{_rep}")
                xbuf = big_pool.tile([128, NSEG, WP], dt, tag="xbuf",
                                     name=f"xbuf_{_rep}")

                if _rep == 0:
                    nc.sync.dma_start(out=wbuf[:], in_=wb.ap())
                    if warmup:
                        wt = wpsum_pool.tile([128, 512], mybir.dt.float32,
                                             tag="wt")
                        for _wi in range(warmup):
                            nc.tensor.matmul(
                                wt[:], wbuf[:, 0:128], wbuf[:, 0:512],
                                start=True, stop=True,
                            )

                # streaming state
                even_done = 0   # rybuf even segs completed (exclusive)
                odd_done = 0
                x_done = 0
                group = 0       # next matmul group to issue (8 groups of 2 sg)

                def issue_group(g):
                    ptiles = []
                    for si in (0, 1):
                        sg = 2 * g + si
                        pt = psum_pool.tile(
                            [128, 4, 128], mybir.dt.float32, tag="ps",
                            name=f"ps_{_rep}_{sg}",
                        )
                        ptiles.append(pt)
                        for bi, (d, v) in enumerate(blocks):
                            lhsT = wbuf[:, bi * 128:(bi + 1) * 128]
                            t0 = 4 * sg + d
                            rhs = xbuf[:, t0:t0 + 4, v:v + 128]
                            nc.tensor.matmul(
                                pt[:], lhsT, rhs,
                                start=(bi == 0), stop=(bi == nblk - 1),
                            )
                    st = stage_pool.tile([128, 2, 512], dt, tag="st",
                                         name=f"st_{_rep}_{g}")
                    nc.scalar.copy(
                        out=st[:, 0],
                        in_=ptiles[0][:].rearrange("p a b -> p (a b)"))
                    # second drain on DVE when it has slack, and always
                    # for the final group so the two tail drains overlap
                    if pool_drain or g == 7:
                        nc.vector.tensor_copy(
                            st[:, 1],
                            ptiles[1][:].rearrange("p a b -> p (a b)"))
                    else:
                        nc.scalar.copy(
                            out=st[:, 1],
                            in_=ptiles[1][:].rearrange("p a b -> p (a b)"))
                    nc.gpsimd.dma_start(
                        out=outv[:, 2 * g:2 * g + 2], in_=st[:])

                c0 = 0
                for ci, c1 in enumerate(ends):
                    # ---- load chunk (abuf on SP queue, bbuf on Act's
                    # queue so the two streams issue in parallel) ----
                    nc.sync.dma_start(out=abuf[:, c0:c1], in_=xav[:, c0:c1])
                    if ship == 2:
                        nc.scalar.dma_start(out=bbuf[:, c0:c1],
                                            in_=xbv[:, c0:c1])

                    # ---- y-blend ----
                    if ship == 1:
                        e0, e1 = even_done, min(c1, NSEG)
                        o0, o1 = odd_done, c1 - 1
                        ln = max(e1 - e0, o1 - o0)
                        scr = scr_pool.tile([128, max(ln, 1), WP], dt,
                                            tag="scry",
                                            name=f"scry_{_rep}_{ci}")
                        if e1 > e0:
                            lo, hi = e0, e1
                            base = abuf[0:64, lo:hi]
                            sc = abuf[64:128, lo:hi]
                            if y_swap:
                                base, sc = sc, base
                            nc.scalar.activation(
                                scr[0:64, 0:hi - lo], sc,
                                mybir.ActivationFunctionType.Copy,
                                scale=float(sy))
                            nc.vector.tensor_tensor(
                                rybuf[0:64, lo:hi], scr[0:64, 0:hi - lo],
                                base, op=mybir.AluOpType.add)
                            even_done = e1
                        if o1 > o0:
                            lo, hi = o0, o1
                            base = abuf[64:128, lo:hi]
                            sc = abuf[0:64, lo + 1:hi + 1]
                            if y_swap:
                                base, sc = sc, base
                            nc.vector.tensor_scalar(
                                scr[64:128, 0:hi - lo], sc, float(sy), None,
                                op0=mybir.AluOpType.mult)
                            nc.vector.tensor_tensor(
                                rybuf[64:128, lo:hi], scr[64:128, 0:hi - lo],
                                base, op=mybir.AluOpType.add)
                            odd_done = o1
                        y_done = min(even_done, odd_done)
                    else:
                        lo, hi = even_done, c1
                        ln = hi - lo
                        scr = scr_pool.tile([128, max(ln, 1), WP], dt,
                                            tag="scry",
                                            name=f"scry_{_rep}_{ci}")
                        ya, yb = abuf[:, lo:hi], bbuf[:, lo:hi]
                        y_sc, y_base = (ya, yb) if y_swap else (yb, ya)
                        nc.scalar.activation(
                            scr[:, 0:ln], y_sc,
                            mybir.ActivationFunctionType.Copy,
                            scale=float(sy))
                        nc.vector.tensor_tensor(
                            rybuf[:, lo:hi], scr[:, 0:ln], y_base,
                            op=mybir.AluOpType.add)
                        even_done = odd_done = c1
                        y_done = c1

                    # ---- x-blend ----
                    if y_done > x_done:
                        lo, hi = x_done, y_done
                        ln = hi - lo
                        scrx = scr_pool.tile([128, ln, 131], dt, tag="scrx",
                                             name=f"scrx_{_rep}_{ci}")
                        r0 = rybuf[:, lo:hi, 0:131]
                        r1 = rybuf[:, lo:hi, 1:132]
                        x_sc, x_base = (r0, r1) if x_swap else (r1, r0)
                        nc.vector.tensor_scalar(
                            scrx[:], x_sc, float(sx), None,
                            op0=mybir.AluOpType.mult)
                        nc.vector.tensor_tensor(
                            xbuf[:, lo:hi, 0:131], scrx[:], x_base,
                            op=mybir.AluOpType.add)
                        x_done = y_done

                    # ---- matmul groups now unlocked ----
                    while group < 8 and x_done >= 8 * group + 9:
                        issue_group(group)
                        group += 1
                while group < 8:
                    issue_group(group)
                    group += 1

    nc.compile()
    return nc


def build_general(plan, repeat=1):
    """Fallback: dense folded conv (up to 12 banded blocks), f32r,
    device-side padding memsets."""
    blocks = plan["blocks"]
    nblk = len(blocks)
    dt_x = mybir.dt.float32r
    ms_cast = lambda ap: ap.bitcast(mybir.dt.float32)  # noqa: E731
    GWP, GNSEG = 132, 66
    nc = bacc.Bacc(
        "TRN2", target_bir_lowering=False, debug=False, enable_asserts=False
    )
    xs = nc.dram_tensor("xs", [CIN, H, W], dt_x, kind="ExternalInput")
    wb = nc.dram_tensor("wb", [128, nblk * 128], dt_x, kind="ExternalInput")
    out = nc.dram_tensor("out", [COUT, H, W], mybir.dt.float32,
                         kind="ExternalOutput")

    with tile.TileContext(nc) as tc:
        with (
            tc.tile_pool(name="const", bufs=1) as const_pool,
            tc.tile_pool(name="psum", bufs=8, space="PSUM") as psum_pool,
            tc.tile_pool(name="stage", bufs=8) as stage_pool,
        ):
            xbuf = const_pool.tile([128, GNSEG * GWP], dt_x, tag="xbuf")
            wbuf = const_pool.tile([128, nblk * 128], dt_x, tag="wbuf")
            xv = xbuf[:].rearrange("p (t c) -> p t c", c=GWP)

            for _rep in range(repeat):
                nc.sync.dma_start(out=wbuf[:], in_=wb.ap())

                nc.gpsimd.memset(ms_cast(xv[:, :, 0:1]), 0.0)
                nc.gpsimd.memset(ms_cast(xv[:, :, 129:132]), 0.0)
                nc.gpsimd.memset(ms_cast(xv[0:64, 0:1, :]), 0.0)
                nc.gpsimd.memset(ms_cast(xv[64:128, 64:65, :]), 0.0)
                nc.gpsimd.memset(ms_cast(xv[:, 65:66, :]), 0.0)

                xap = xs.ap()
                for t0, t1 in ((1, 17), (17, 33), (33, 49), (49, 64)):
                    dram = xap[:, 2 * t0 - 1:2 * t1 - 1, :].rearrange(
                        "ci (t j) x -> j ci t x", j=2
                    )
                    for j in (0, 1):
                        nc.sync.dma_start(
                            out=xv[j * 64:(j + 1) * 64, t0:t1, 1:129],
                            in_=dram[j],
                        )
                nc.sync.dma_start(out=xv[64:128, 0:1, 1:129], in_=xap[:, 0:1, :])
                nc.sync.dma_start(out=xv[0:64, 64:65, 1:129],
                                  in_=xap[:, 127:128, :])

                out_ap = out.ap().rearrange(
                    "co (s g i) x -> s i co g x", g=4, i=2)

                for half in range(2):
                    ptiles = [
                        psum_pool.tile(
                            [128, 4, 128], mybir.dt.float32, tag="ps",
                            name=f"ps_{_rep}_{half}_{k}",
                        )
                        for k in range(8)
                    ]
                    for bi, (d, v) in enumerate(blocks):
                        lhsT = wbuf[:, bi * 128:(bi + 1) * 128]
                        first = bi == 0
                        last = bi == nblk - 1
                        for sg in range(half * 8, half * 8 + 8):
                            t0 = 4 * sg + d
                            rhs = xv[:, t0:t0 + 4, v:v + 128]
                            nc.tensor.matmul(
                                ptiles[sg % 8][:], lhsT, rhs,
                                start=first, stop=last,
                            )
                    for sg in range(half * 8, half * 8 + 8):
                        st = stage_pool.tile(
                            [128, 4, 128], mybir.dt.float32, tag="st",
                            name=f"st_{_rep}_{half}_{sg}",
                        )
                        nc.vector.tensor_copy(st[:], ptiles[sg % 8][:])
                        for i in (0, 1):
                            nc.sync.dma_start(
                                out=out_ap[sg][i],
                                in_=st[i * 64:(i + 1) * 64],
                            )

    nc.compile()
    return nc


_CACHE = {}


def _get_nc(plan, repeat=1):
    key = (plan["key"], repeat, SHIP, WARMUP, POOL_DRAIN, CHUNKS_ENV)
    if key not in _CACHE:
        builder = build_fast if plan["mode"] == "fast" else build_general
        _CACHE[key] = builder(plan, repeat)
    return _CACHE[key]


# --------------------------------------------------------------------
# host-side staging
# --------------------------------------------------------------------

def make_in_maps(x, plan):
    """Per-core input dicts. Fast path: pack each image into the
    row-pair SBUF layout(s) (bf16, zeros baked in)."""
    x = np.asarray(x)
    if plan["mode"] == "general":
        Whc = np.ascontiguousarray(plan["Wh"].astype(np.float32))
        return [
            {"xs": np.ascontiguousarray(x[b].astype(np.float32)), "wb": Whc}
            for b in range(B)
        ]
    Whc = np.ascontiguousarray(plan["Wh"].astype(BF16))
    xb16 = x.astype(BF16)
    in_maps = []
    for b in range(B):
        xp = np.zeros((CIN, 132, 132), dtype=BF16)
        xp[:, 1:129, 1:129] = xb16[b]
        if SHIP == 1:
            A = np.ascontiguousarray(
                xp.reshape(CIN, 66, 2, 132)
                .transpose(2, 0, 1, 3).reshape(128, 66 * WP))
            in_maps.append({"xa": A, "wb": Whc})
        else:
            A = np.ascontiguousarray(
                xp[:, 0:130, :].reshape(CIN, NSEG, 2, 132)
                .transpose(2, 0, 1, 3).reshape(128, NSEG * WP))
            Bm = np.ascontiguousarray(
                xp[:, 1:131, :].reshape(CIN, NSEG, 2, 132)
                .transpose(2, 0, 1, 3).reshape(128, NSEG * WP))
            in_maps.append({"xa": A, "xb2": Bm, "wb": Whc})
    return in_maps


def unpack_out(res, plan):
    """Per-core output dict -> [COUT, H, W] float32."""
    arr = np.asarray(res["out"])
    if plan["mode"] == "general":
        return arr.astype(np.float32)
    return np.ascontiguousarray(
        arr.reshape(2, 64, 16, 4, 128).transpose(1, 2, 3, 0, 4)
        .reshape(COUT, H, W)).astype(np.float32)


def kernel(x, weights, tap_offsets):
    x = np.asarray(x)
    assert x.shape == (B, CIN, H, W)
    plan = plan_from_inputs(weights, tap_offsets)
    nc = _get_nc(plan)
    in_maps = make_in_maps(x, plan)
    res = run_bass_kernel_spmd(nc, in_maps, list(range(NCORES)), trace=False)
    outs = [unpack_out(res.results[c], plan) for c in range(NCORES)]
    return np.stack(outs, 0).astype(np.float32)
